# revision 1
# baseline (speedup 1.0000x reference)
"""2D Gaussian Splatting on 8 Trainium2 NeuronCores.

Strategy: shard pixels across cores (embarrassingly parallel); each core owns
32 image rows = 16 tiles of 32x16 px (512 px each). Host-side 3.5-sigma
bounding-box culling gives <=127 gaussians per tile (measured max 121 for this
input distribution), so the front-to-back transmittance scan per tile is a
single 127x127 triangular matmul.

Per tile (phase-ordered across tiles so the ACT table loads only 3x):
  z+b  = -q/2 + bias       K=22 fp32r matmul (hi/lo split -> fp32 exact;
                           bias = log opac + const coef folded in as rows)
  alpha = exp(z + b)       ACT Exp, batched 2 tiles/op from PSUM
  lom  = ln(1 - alpha)     ACT Ln, batched 4 tiles/op
  logT = tri @ lom         K=127 triangular fp32r matmul (exclusive scan)
  T    = exp(logT)         ACT Exp, batched 2 tiles/op from PSUM
  w    = alpha * T         DVE multiply -> bf16
  img  = colors^T @ w      K=127 bf16 matmul -> [1, 512] at psum row 32*(t%4)
"""

import math
import numpy as np

W = 256
H = 256
TILE_W = 32
TILE_H = 16
F = TILE_W * TILE_H      # 512 pixels per tile
NTX = W // TILE_W        # 8 tile cols
NTY = H // TILE_H        # 16 tile rows
NT = 16                  # tiles per core (2 tile rows x 8 tile cols)
MG = 127                 # max gaussians per tile
NB = 5                   # basis monomials: x'^2, x'y', y'^2, x', y'
KQ = 4 * NB + 2          # hi/lo coef x hi/lo basis pairs + bias hi/lo rows
N_CORES = 8
PAD_BIAS = -60.0         # exp(-60) == 0 for padded gaussian slots
SIGMA_K = 3.5

# fp32r q-matmul const tensor layout (columns): basis | coefs | tri
# full 127-row height for full DMA bandwidth; 3 column-chunks on parallel
# queues; chunk 0 = basis + tiles 0-3 so phase 1 starts earliest
C_BASIS = 0
C_COEF = F
C_TRI = C_COEF + NT * 128
QB_W = C_TRI + MG
QB_H = 128
QB_CH = [
    (0, F + 4 * 128),                      # basis + tiles 0-3
    (F + 4 * 128, 8 * 128),                # tiles 4-11
    (F + 12 * 128, 4 * 128 + MG),          # tiles 12-15 + tri
]


def _round_fp32r(a):
    """Round float32 array to fp32r (RNE, drop low 12 mantissa bits)."""
    b = np.asarray(a, np.float32).view(np.uint32).astype(np.uint64)
    r = (b + 0x7FF + ((b >> 12) & 1)) & 0xFFFFF000
    return r.astype(np.uint32).view(np.float32)


def _split_fp32r(a):
    """Split float32 array into fp32r hi + lo with hi+lo ~= a to ~2^-24."""
    a = np.asarray(a, np.float32)
    hi = _round_fp32r(a)
    lo = _round_fp32r(a - hi)
    return hi, lo


def _setup_act_tables():
    """Point walrus at a reordered act_info.json whose first ln/exp set is
    the combined natural_log_exp_and_others, so the phase-ordered ACT stream
    (exp... ln... exp...) needs a single table load instead of three."""
    import glob
    import json
    import os
    import tempfile

    if os.environ.get("BASS_ACT_ROOT_JSON_PATH"):
        return
    try:
        import neuronxcc

        root = os.path.join(
            os.path.dirname(neuronxcc.__file__), "pwp", "pwp_bin_trainium"
        )
        src = os.path.join(root, "act_info.json")
        d = json.load(open(src))
        sets = d["act_func_sets"]
        comb = [x for x in sets if x["name"] == "natural_log_exp_and_others"]
        if not comb:
            return
        d["act_func_sets"] = comb + [x for x in sets if x is not comb[0]]
        tmpd = tempfile.mkdtemp(prefix="actroot_")
        for f in os.listdir(root):
            if f != "act_info.json":
                os.symlink(os.path.join(root, f), os.path.join(tmpd, f))
        dst = os.path.join(tmpd, "act_info.json")
        json.dump(d, open(dst, "w"))
        os.environ["BASS_ACT_ROOT_JSON_PATH"] = dst
    except Exception:
        pass


def _build_nc():
    import concourse.bacc as bacc
    import concourse.mybir as mybir
    from concourse.tile import TileContext

    f32 = mybir.dt.float32
    f32r = mybir.dt.float32r
    bf16 = mybir.dt.bfloat16
    AF = mybir.ActivationFunctionType

    nc = bacc.Bacc("TRN2", target_bir_lowering=False, debug=False)
    qb_d = nc.declare_dram_parameter("qb", [QB_H, QB_W], f32r, isOutput=False)
    colors_d = nc.declare_dram_parameter("colors", [128, NT], bf16, isOutput=False)
    out_d = nc.declare_dram_parameter("out", [4, 4 * F], f32, isOutput=True)

    with TileContext(nc) as tc:
        with (
            tc.tile_pool(name="const", bufs=1) as cpool,
            tc.tile_pool(name="alpha", bufs=4) as apool,
            tc.tile_pool(name="lom", bufs=4) as lpool,
            tc.tile_pool(name="tt", bufs=3) as tpool,
            tc.tile_pool(name="w", bufs=6) as wpool,
            tc.tile_pool(name="ps", bufs=2, space="PSUM") as pspool,
            tc.tile_pool(name="psi", bufs=1, space="PSUM") as ipool,
        ):
            colorst = cpool.tile([128, NT], bf16)
            nc.sync.dma_start(colorst[:], colors_d[:])
            qbt = cpool.tile([QB_H, QB_W], f32r)
            # two chunks on two issuing queues (Sync + ACT) -> parallel DMAs
            ch = C_COEF + 8 * 128
            nc.sync.dma_start(qbt[:, 0:ch], qb_d[:, 0:ch])
            nc.scalar.dma_start(qbt[:, ch:QB_W], qb_d[:, ch:QB_W])
            tri = qbt[0:MG, C_TRI : C_TRI + MG]
            out_sb = cpool.tile([97, 4 * F], f32)

            # dummy ops: ACT/PE observe the input DMAs (and ACT loads the exp
            # table) on dedicated instructions so real ops carry <=1 wait each
            dummy = cpool.tile([1, 1], f32)
            nc.scalar.activation(dummy[:], colorst[0:1, 0:1], AF.Exp, bias=0.0)

            # persistent img banks: tile t -> bank t//4, partition 32*(t%4);
            # no slot reuse, so no cross-engine WAR waits on any matmul
            psi_banks = [
                ipool.tile([97, F], f32, name=f"psib{b}", tag=f"psi{b}")
                for b in range(4)
            ]
            nc.tensor.matmul(
                psi_banks[0][0:1, 0:1],
                colorst[0:1, 0:1],
                colorst[0:1, 0:1],
                start=True,
                stop=True,
            )
            nc.tensor.matmul(
                psi_banks[0][0:2, 0:126],
                qbt[0:1, C_TRI : C_TRI + 2],
                qbt[0:1, C_TRI : C_TRI + 126],
                start=True,
                stop=True,
            )

            # PE pre-warm: ~3.5us of dummy matmuls during the input DMAs so
            # the HAM clock gate opens before the real work starts
            warm = cpool.tile([128, F], bf16)
            nc.gpsimd.memset(warm[:], 0.0)
            for _ in range(10):
                nc.tensor.matmul(
                    psi_banks[1][0:1, :],
                    warm[:, 0:1],
                    warm[:],
                    start=True,
                    stop=True,
                )

            # phase 1 (exp table): z = -q/2 + bias via matmul; alpha = exp(z)
            alpha_grps = [
                apool.tile([MG, 4 * F], f32, name=f"ag{i}", tag="ag")
                for i in range(4)
            ]
            for g in range(8):  # pairs of tiles
                ps = pspool.tile([MG, 2 * F], f32, name="psq")
                for j in range(2):
                    t = 2 * g + j
                    nc.tensor.matmul(
                        ps[:, j * F : (j + 1) * F],
                        qbt[0:KQ, C_COEF + t * 128 : C_COEF + t * 128 + MG],
                        qbt[0:KQ, C_BASIS : C_BASIS + F],
                        start=True,
                        stop=True,
                    )
                nc.scalar.activation(
                    alpha_grps[g // 2][:, (g % 2) * 2 * F : ((g % 2) + 1) * 2 * F],
                    ps[:],
                    AF.Exp,
                    bias=0.0,
                )

            # phase 2 (ln table): lom = ln(1 - alpha), 4 tiles per op
            lom_grps = []
            for i in range(4):
                lom = lpool.tile([MG, 4 * F], f32r, name=f"lg{i}", tag="lg")
                nc.scalar.activation(
                    lom[:], alpha_grps[i][:], AF.Ln, bias=1.0, scale=-1.0
                )
                lom_grps.append(lom)

            # PE fillers: keep the HAM clock gate open across the ln phase
            # (the PE would otherwise idle >5us here and re-throttle)
            for _ in range(32):
                nc.tensor.matmul(
                    psi_banks[1][0:1, :],
                    warm[:, 0:1],
                    warm[:],
                    start=True,
                    stop=True,
                )

            # phase 3 (exp table): triangular scan, T = exp(logT),
            # w = alpha * T, img = colors^T @ w
            def consume(g, Tt):
                for j in range(2):
                    t = 2 * g + j
                    w = wpool.tile([MG, F], bf16, name="wt", tag="wt")
                    nc.vector.tensor_mul(
                        w[:],
                        alpha_grps[t // 4][:, (t % 4) * F : (t % 4 + 1) * F],
                        Tt[:, j * F : (j + 1) * F],
                    )
                    b, jj = divmod(t, 4)
                    nc.tensor.matmul(
                        psi_banks[b][32 * jj : 32 * jj + 1, :],
                        colorst[0:MG, t : t + 1],
                        w[:],
                        start=True,
                        stop=True,
                        tile_position=(0, 32 * jj),
                    )
                    if jj == 3:
                        nc.vector.tensor_copy(
                            out_sb[:, b * F : (b + 1) * F], psi_banks[b][:]
                        )
                        nc.sync.dma_start(
                            out_d[:, b * F : (b + 1) * F],
                            out_sb[0:97:32, b * F : (b + 1) * F],
                        )

            # consume lags 2 groups behind so img matmuls (which wait on DVE
            # multiplies) never block the tri-matmul stream in the PE queue
            pending = []
            for g in range(8):
                ps = pspool.tile([MG, 2 * F], f32, name="psq")
                for j in range(2):
                    t = 2 * g + j
                    nc.tensor.matmul(
                        ps[:, j * F : (j + 1) * F],
                        tri,
                        lom_grps[t // 4][:, (t % 4) * F : (t % 4 + 1) * F],
                        start=True,
                        stop=True,
                    )
                Tt = tpool.tile([MG, 2 * F], f32, name="Tt", tag="Tt")
                nc.scalar.activation(Tt[:], ps[:], AF.Exp, bias=0.0)
                pending.append((g, Tt))
                if len(pending) > 2:
                    consume(*pending.pop(0))
            for item in pending:
                consume(*item)

    nc.compile()
    return nc


_NC = None
LAST_RESULT = None


def _get_nc():
    global _NC
    if _NC is None:
        _NC = _build_nc()
    return _NC


def _prep_inputs(means, quats, scales, rgbs, opacities):
    """Host-side projection + per-tile culling; returns per-core input maps."""
    import ml_dtypes

    means = np.asarray(means, np.float64)
    quats = np.asarray(quats, np.float64)
    scales = np.asarray(scales, np.float64)
    rgbs = np.asarray(rgbs, np.float64)
    opacities = np.asarray(opacities, np.float64)

    c = np.cos(quats)
    s = np.sin(quats)
    sx2 = scales[:, 0] ** 2
    sy2 = scales[:, 1] ** 2
    a11 = c * c * sx2 + s * s * sy2
    a12 = c * s * (sx2 - sy2)
    a22 = s * s * sx2 + c * c * sy2
    det = a11 * a22 - a12 * a12
    ia = a22 / det
    ib = -a12 / det
    ic = a11 / det

    logopac = -np.logaddexp(0.0, -opacities)        # log(sigmoid(o))
    colors = 1.0 / (1.0 + np.exp(-rgbs[:, 0]))      # sigmoid, C=1

    rx = SIGMA_K * np.sqrt(a11)
    ry = SIGMA_K * np.sqrt(a22)
    x0g, x1g = means[:, 0] - rx, means[:, 0] + rx
    y0g, y1g = means[:, 1] - ry, means[:, 1] + ry

    tri = np.triu(np.ones((MG, MG), np.float32), 1)  # lhsT[j,i]=1 iff j<i

    # basis in tile-local coords (identical for every tile)
    fx = (np.arange(F) % TILE_W).astype(np.float64) - (TILE_W - 1) / 2.0
    fy = (np.arange(F) // TILE_W).astype(np.float64) - (TILE_H - 1) / 2.0
    basis5 = np.stack([fx * fx, fx * fy, fy * fy, fx, fy]).astype(np.float32)
    bhi, blo = _split_fp32r(basis5)
    basis = np.concatenate(
        [bhi, blo, bhi, blo, np.ones((2, F), np.float32)]
    )  # [KQ, F]

    in_maps = []
    for core in range(N_CORES):
        qb = np.zeros((QB_H, QB_W), np.float32)
        qb[0:KQ, C_BASIS : C_BASIS + F] = basis
        qb[0:MG, C_TRI : C_TRI + MG] = tri
        colarr = np.zeros((128, NT), ml_dtypes.bfloat16)
        for t in range(NT):
            tyl, tx = divmod(t, NTX)
            ty = core * 2 + tyl
            X0, X1 = tx * TILE_W, (tx + 1) * TILE_W
            Y0, Y1 = ty * TILE_H, (ty + 1) * TILE_H
            idx = np.nonzero((x1g >= X0) & (x0g <= X1) & (y1g >= Y0) & (y0g <= Y1))[0]
            if len(idx) > MG:
                for k in (3.25, 3.0, 2.75, 2.5, 2.25, 2.0):
                    fac = k / SIGMA_K
                    m = (
                        (means[idx, 0] + fac * rx[idx] >= X0)
                        & (means[idx, 0] - fac * rx[idx] <= X1)
                        & (means[idx, 1] + fac * ry[idx] >= Y0)
                        & (means[idx, 1] - fac * ry[idx] <= Y1)
                    )
                    if m.sum() <= MG:
                        idx = idx[m]
                        break
                else:
                    idx = idx[:MG]
            k = len(idx)
            cx = X0 + (TILE_W - 1) / 2.0 + 0.5   # center of pixel-center range
            cy = Y0 + (TILE_H - 1) / 2.0 + 0.5
            mx = means[idx, 0] - cx
            my = means[idx, 1] - cy
            iag, ibg, icg = ia[idx], ib[idx], ic[idx]
            coef5 = np.zeros((NB, 128), np.float32)
            coef5[0, :k] = -0.5 * iag
            coef5[1, :k] = -ibg
            coef5[2, :k] = -0.5 * icg
            coef5[3, :k] = iag * mx + ibg * my
            coef5[4, :k] = ibg * mx + icg * my
            bias = np.full(128, PAD_BIAS, np.float32)
            bias[:k] = (
                logopac[idx]
                - 0.5 * iag * mx * mx - ibg * mx * my - 0.5 * icg * my * my
            )
            chi, clo = _split_fp32r(coef5)
            bias_hi, bias_lo = _split_fp32r(bias)
            qb[0:KQ, C_COEF + t * 128 : C_COEF + (t + 1) * 128] = (
                np.concatenate([chi, chi, clo, clo, bias_hi[None], bias_lo[None]])
            )
            colarr[:k, t] = colors[idx].astype(ml_dtypes.bfloat16)
        in_maps.append({"qb": qb, "colors": colarr})
    return in_maps


def kernel(means, quats, scales, rgbs, opacities):
    global LAST_RESULT
    from concourse.bass_utils import run_bass_kernel_spmd

    in_maps = _prep_inputs(means, quats, scales, rgbs, opacities)
    nc = _get_nc()
    res = run_bass_kernel_spmd(nc, in_maps, list(range(N_CORES)))
    LAST_RESULT = res

    img = np.zeros((H, W), np.float32)
    for core in range(N_CORES):
        out = np.asarray(res.results[core]["out"]).reshape(4, 4, TILE_H, TILE_W)
        for t in range(NT):
            b, j = divmod(t, 4)
            tyl, tx = divmod(t, NTX)
            ty = core * 2 + tyl
            img[
                ty * TILE_H : (ty + 1) * TILE_H, tx * TILE_W : (tx + 1) * TILE_W
            ] = out[j, b]
    return img[None, None].astype(np.float32)



# revision 10
# speedup vs baseline: 1.6886x; 1.6886x over previous
"""2D Gaussian Splatting on 8 Trainium2 NeuronCores — layout-B cumprod design.

Pixels live on partitions: each pixel-tile is 16x8 = 128 px (one column set
of the SBUF). Per tile, the culled gaussian list (3.5-sigma bbox, global
index order) occupies a variable-length run of free-dim columns, prefixed by
one spacer column; 512 tiles are greedily bin-packed across the 8 cores so
every core's packed width is ~equal (GCAP cols).

Per core, single pass over the packed axis (chunks of <=512 cols for PSUM):
  zb  = basisT @ coefs (+ logopac bias rows)      fp32r matmul K=9
  zbc = basisT @ coefs (+ (logopac+ln c) rows)    second matmul, shared rhs
  alpha = Exp(zb)   [ACT]     AC = Exp(zbc) -> bf16 [ACT]
  om = 1 - alpha    [GP tensor_scalar]
  s  = segmented cumprod: scan state' = max(om*state, d1p)  [DVE]
       (spacer cols have zb=0 -> om=0, d1p=1 -> exact reset to 1)
  wc = AC * s_shifted_one_col  [DVE, bf16]
  sums = running cumsum of wc (scan state' = wc + state)  [DVE/GP split]
  out = sums (f32) DMA'd whole; host extracts per-tile sums by differencing
        sums[end_col] - sums[spacer_col] (spacer junk cancels).
"""

import math
import numpy as np

W = 256
H = 256
TW = 16            # pixel tile width
TH = 8             # pixel tile height
NTX = W // TW      # 16
NTY = H // TH      # 32
NTILES = NTX * NTY # 512
N_CORES = 8
SIGMA_K = 3.5
KQ = 9             # 5 coef rows + bias hi/lo + colorbias hi/lo
CW = 512           # chunk width (one PSUM bank of f32)


def _round_fp32r(a):
    b = np.asarray(a, np.float32).view(np.uint32).astype(np.uint64)
    r = (b + 0x7FF + ((b >> 12) & 1)) & 0xFFFFF000
    return r.astype(np.uint32).view(np.float32)


def _split_fp32r(a):
    a = np.asarray(a, np.float32)
    hi = _round_fp32r(a)
    lo = _round_fp32r(a - hi)
    return hi, lo


def _build_nc(gcap, debug=False):
    import concourse.bacc as bacc
    import concourse.mybir as mybir
    from concourse.tile import TileContext

    f32 = mybir.dt.float32
    f32r = mybir.dt.float32r
    bf16 = mybir.dt.bfloat16
    AF = mybir.ActivationFunctionType
    OP = mybir.AluOpType

    chunks = []
    c0 = 0
    while c0 < gcap:
        cw = min(CW, gcap - c0)
        chunks.append((c0, cw))
        c0 += cw
    nch = len(chunks)

    nc = bacc.Bacc("TRN2", target_bir_lowering=False, debug=False)
    rhs_d = nc.declare_dram_parameter("rhs", [KQ, gcap], f32r, isOutput=False)
    lhsT_d = nc.declare_dram_parameter("lhsT", [KQ, 256], f32r, isOutput=False)
    d1p_d = nc.declare_dram_parameter("d1p", [128, gcap], bf16, isOutput=False)
    out_d = nc.declare_dram_parameter("out", [128, gcap], f32, isOutput=True)
    if debug:
        dbg_a = nc.declare_dram_parameter("dbg_a", [128, gcap], f32, isOutput=True)
        dbg_om = nc.declare_dram_parameter("dbg_om", [128, gcap], f32, isOutput=True)
        dbg_s = nc.declare_dram_parameter("dbg_s", [128, gcap + 1], f32, isOutput=True)
        dbg_wc = nc.declare_dram_parameter("dbg_wc", [128, gcap], f32, isOutput=True)
        dbg_ac = nc.declare_dram_parameter("dbg_ac", [128, gcap], f32, isOutput=True)

    with TileContext(nc) as tc:
        with (
            tc.tile_pool(name="const", bufs=1) as cpool,
            tc.tile_pool(name="ps", bufs=3, space="PSUM") as pspool,
            tc.tile_pool(name="pw", bufs=1, space="PSUM") as pwpool,
        ):
            rhs_t = cpool.tile([KQ, gcap], f32r)
            lhsT_t = cpool.tile([KQ, 256], f32r)
            d1p_t = cpool.tile([128, gcap], bf16)
            abuf = cpool.tile([128, gcap], f32)
            acbuf = cpool.tile([128, gcap], bf16)
            ombuf = cpool.tile([128, gcap], f32)
            sbufS = cpool.tile([128, gcap + 1], bf16)
            wcbuf = cpool.tile([128, gcap], bf16)
            sumbuf = cpool.tile([128, gcap], f32)
            zeros = cpool.tile([128, CW], f32)
            warm = cpool.tile([128, CW], bf16)

            # input DMAs: d1p split over 4 queues, rhs/lhsT on sync
            nc.sync.dma_start(lhsT_t[:], lhsT_d[:])
            nc.sync.dma_start(rhs_t[:], rhs_d[:])
            q = gcap // 4
            nc.sync.dma_start(d1p_t[:, 0:q], d1p_d[:, 0:q])
            nc.scalar.dma_start(d1p_t[:, q : 2 * q], d1p_d[:, q : 2 * q])
            nc.sync.dma_start(d1p_t[:, 2 * q : 3 * q], d1p_d[:, 2 * q : 3 * q])
            nc.scalar.dma_start(d1p_t[:, 3 * q : gcap], d1p_d[:, 3 * q : gcap])

            nc.gpsimd.memset(zeros[:], 0.0)
            nc.gpsimd.memset(sbufS[:, 0:1], 0.0)
            nc.gpsimd.memset(warm[:], 0.0)

            # ACT exp-table preload during input DMA
            dummy = cpool.tile([1, 1], f32)
            nc.scalar.activation(dummy[:], zeros[0:1, 1:2], AF.Exp, bias=0.0)

            # PE p-state warmup during input DMA
            wps = pwpool.tile([1, CW], f32, name="warmps")
            for _ in range(6):
                nc.tensor.matmul(
                    wps[:], warm[:, 0:1], warm[:], start=True, stop=True
                )

            for ci, (c0, cw) in enumerate(chunks):
                sl = slice(c0, c0 + cw)
                psA = pspool.tile([128, cw], f32, name="psA")
                psB = pspool.tile([128, cw], f32, name="psB")
                nc.tensor.matmul(
                    psA[:], lhsT_t[:, 0:128], rhs_t[:, sl], start=True, stop=True
                )
                nc.tensor.matmul(
                    psB[:], lhsT_t[:, 128:256], rhs_t[:, sl], start=True, stop=True
                )
                nc.scalar.activation(abuf[:, sl], psA[:], AF.Exp, bias=0.0)
                nc.scalar.activation(acbuf[:, sl], psB[:], AF.Exp, bias=0.0)
                nc.gpsimd.tensor_scalar(
                    ombuf[:, sl], abuf[:, sl], -1.0, 1.0, OP.mult, OP.add
                )
                nc.vector.tensor_tensor_scan(
                    sbufS[:, c0 + 1 : c0 + cw + 1],
                    ombuf[:, sl],
                    d1p_t[:, sl],
                    0.0 if ci == 0 else sbufS[:, c0 : c0 + 1],
                    OP.mult,
                    OP.max,
                )
                nc.vector.tensor_mul(wcbuf[:, sl], acbuf[:, sl], sbufS[:, sl])
                nc.vector.tensor_tensor_scan(
                    sumbuf[:, sl],
                    wcbuf[:, sl],
                    zeros[:, 0:cw],
                    0.0 if ci == 0 else sumbuf[:, c0 - 1 : c0],
                    OP.add,
                    OP.add,
                )
                nc.sync.dma_start(out_d[:, sl], sumbuf[:, sl])

            if debug:
                dbg_af = cpool.tile([128, gcap], f32)
                dbg_omf = cpool.tile([128, gcap], f32)
                dbg_sf = cpool.tile([128, gcap + 1], f32)
                dbg_wcf = cpool.tile([128, gcap], f32)
                dbg_acf = cpool.tile([128, gcap], f32)
                nc.vector.tensor_copy(dbg_af[:], abuf[:])
                nc.vector.tensor_copy(dbg_omf[:], ombuf[:])
                nc.vector.tensor_copy(dbg_sf[:], sbufS[:])
                nc.vector.tensor_copy(dbg_wcf[:], wcbuf[:])
                nc.vector.tensor_copy(dbg_acf[:], acbuf[:])
                nc.sync.dma_start(dbg_a[:], dbg_af[:])
                nc.sync.dma_start(dbg_om[:], dbg_omf[:])
                nc.sync.dma_start(dbg_s[:], dbg_sf[:])
                nc.sync.dma_start(dbg_wc[:], dbg_wcf[:])
                nc.sync.dma_start(dbg_ac[:], dbg_acf[:])

    nc.compile()
    return nc


_NC_CACHE = {}
LAST_RESULT = None


def _get_nc(gcap):
    if gcap not in _NC_CACHE:
        _NC_CACHE[gcap] = _build_nc(gcap)
    return _NC_CACHE[gcap]


def _prep_inputs(means, quats, scales, rgbs, opacities):
    """Cull + pack per core. Returns (in_maps, extract, gcap)."""
    import ml_dtypes

    means = np.asarray(means, np.float64)
    quats = np.asarray(quats, np.float64)
    scales = np.asarray(scales, np.float64)
    rgbs = np.asarray(rgbs, np.float64)
    opacities = np.asarray(opacities, np.float64)

    c = np.cos(quats)
    s = np.sin(quats)
    sx2 = scales[:, 0] ** 2
    sy2 = scales[:, 1] ** 2
    a11 = c * c * sx2 + s * s * sy2
    a12 = c * s * (sx2 - sy2)
    a22 = s * s * sx2 + c * c * sy2
    det = a11 * a22 - a12 * a12
    ia = a22 / det
    ib = -a12 / det
    ic = a11 / det
    logopac = -np.logaddexp(0.0, -opacities)
    colors = 1.0 / (1.0 + np.exp(-rgbs[:, 0]))
    lnc = np.log(colors)
    rx = SIGMA_K * np.sqrt(a11)
    ry = SIGMA_K * np.sqrt(a22)
    x0g, x1g = means[:, 0] - rx, means[:, 0] + rx
    y0g, y1g = means[:, 1] - ry, means[:, 1] + ry

    # cull per tile
    tile_idx = []
    for t in range(NTILES):
        ty, tx = divmod(t, NTX)
        X0, X1 = tx * TW, (tx + 1) * TW
        Y0, Y1 = ty * TH, (ty + 1) * TH
        idx = np.nonzero(
            (x1g >= X0) & (x0g <= X1) & (y1g >= Y0) & (y0g <= Y1)
        )[0]
        tile_idx.append(idx)

    # greedy balance tiles -> cores by packed length (L+1)
    order = sorted(range(NTILES), key=lambda t: -len(tile_idx[t]))
    loads = [0] * N_CORES
    assign = [[] for _ in range(N_CORES)]
    for t in order:
        core = min(range(N_CORES), key=lambda k: loads[k])
        assign[core].append(t)
        loads[core] += len(tile_idx[t]) + 1
    gcap = (max(loads) + 63) & ~63

    # basis (tile-local pixel coords), single fp32r
    fx = (np.arange(128) % TW).astype(np.float64) - (TW - 1) / 2.0
    fy = (np.arange(128) // TW).astype(np.float64) - (TH - 1) / 2.0
    basis5 = _round_fp32r(np.stack([fx * fx, fx * fy, fy * fy, fx, fy]))
    lhsT = np.zeros((KQ, 256), np.float32)
    lhsT[0:5, 0:128] = basis5
    lhsT[5, 0:128] = 1.0
    lhsT[6, 0:128] = 1.0
    lhsT[0:5, 128:256] = basis5
    lhsT[7, 128:256] = 1.0
    lhsT[8, 128:256] = 1.0

    in_maps = []
    extract = []   # per core: list of (tile_id, spacer_col, end_col)
    for core in range(N_CORES):
        rhs = np.zeros((KQ, gcap), np.float32)
        d1row = np.zeros(gcap, np.float32)
        ext = []
        col = 0
        for t in assign[core]:
            idx = tile_idx[t]
            k = len(idx)
            d1row[col] = 1.0
            spacer = col
            col += 1
            if k:
                ty, tx = divmod(t, NTX)
                cx = tx * TW + TW / 2.0
                cy = ty * TH + TH / 2.0
                mx = means[idx, 0] - cx
                my = means[idx, 1] - cy
                iag, ibg, icg = ia[idx], ib[idx], ic[idx]
                rhs[0, col : col + k] = _round_fp32r(-0.5 * iag)
                rhs[1, col : col + k] = _round_fp32r(-ibg)
                rhs[2, col : col + k] = _round_fp32r(-0.5 * icg)
                rhs[3, col : col + k] = _round_fp32r(iag * mx + ibg * my)
                rhs[4, col : col + k] = _round_fp32r(ibg * mx + icg * my)
                bias = logopac[idx] - 0.5 * (
                    iag * mx * mx + 2 * ibg * mx * my + icg * my * my
                )
                bh, bl = _split_fp32r(bias)
                bch, bcl = _split_fp32r(bias + lnc[idx])
                rhs[5, col : col + k] = bh
                rhs[6, col : col + k] = bl
                rhs[7, col : col + k] = bch
                rhs[8, col : col + k] = bcl
                col += k
            ext.append((t, spacer, col - 1 if k else spacer))
        d1p = np.broadcast_to(
            d1row.astype(ml_dtypes.bfloat16), (128, gcap)
        ).copy()
        in_maps.append({"rhs": rhs, "lhsT": lhsT, "d1p": d1p})
        extract.append(ext)
    return in_maps, extract, gcap


def _assemble(results, extract):
    img = np.zeros((H, W), np.float32)
    for core in range(N_CORES):
        sums = np.asarray(results[core]["out"], np.float32)  # [128, gcap]
        for t, b, e in extract[core]:
            ty, tx = divmod(t, NTX)
            colv = sums[:, e] - sums[:, b]
            img[ty * TH : (ty + 1) * TH, tx * TW : (tx + 1) * TW] = colv.reshape(
                TH, TW
            )
    return img[None, None].astype(np.float32)


def kernel(means, quats, scales, rgbs, opacities):
    global LAST_RESULT
    from concourse.bass_utils import run_bass_kernel_spmd

    in_maps, extract, gcap = _prep_inputs(means, quats, scales, rgbs, opacities)
    nc = _get_nc(gcap)
    res = run_bass_kernel_spmd(nc, in_maps, list(range(N_CORES)))
    LAST_RESULT = res
    return _assemble(res.results, extract)


# revision 11
# speedup vs baseline: 2.0162x; 1.1940x over previous
"""2D Gaussian Splatting on 8 Trainium2 NeuronCores — layout-B cumprod design.

Pixels live on partitions: each pixel-tile is 16x8 = 128 px. Per tile, the
culled gaussian list (3.5-sigma bbox, global index order) occupies a run of
free-dim columns: [spacer, g0..g_{L-1}, pads]. The 512 tiles are globally
sorted by gaussian count and dealt round-robin to the 8 cores, so every core
holds 64 tiles in 8 buckets of 8 segments; bucket heights L_j are global
maxima, making the packed geometry identical across cores (one SPMD program).

Single pass over the packed axis (chunks of 512 cols for PSUM):
  zb  = basisT @ coefs (+ logopac bias rows)      fp32r matmul K=9
  zbc = basisT @ coefs (+ (logopac+ln c) rows)    second matmul, shared rhs
        (spacer/pad cols: zb bias 0 -> alpha=1, om=0; zbc bias -60 -> AC=0)
  alpha = Exp(zb) [ACT f32]     AC = Exp(zbc) [ACT bf16]
  om = 1 - alpha  [GP tensor_scalar, bf16 out]
  s  = segmented cumprod: scan state' = max(om*state, d1p) [DVE, bf16]
       d1p built on-device: memset 0 + strided memset 1.0 at spacer cols
  wc = AC * s_shifted_one_col  [DVE bf16]  (0 at spacers/pads since AC=0)
  img column = per-bucket 3D tensor_reduce over segments  [DVE]
  out [128, 64] f32 -> host places each column as a 16x8 pixel block.
"""

import math
import numpy as np

W = 256
H = 256
TW = 16            # pixel tile width
TH = 8             # pixel tile height
NTX = W // TW      # 16
NTY = H // TH      # 32
NTILES = NTX * NTY # 512
N_CORES = 8
NT_CORE = NTILES // N_CORES   # 64 tiles per core
NSEG = 8                      # segments per bucket
NBUCK = NT_CORE // NSEG       # 8 buckets
SIGMA_K = 3.5
KQ = 9             # 5 coef rows + bias hi/lo + colorbias hi/lo
CW = 512           # chunk width (one PSUM bank of f32)
NEG = -60.0        # exp(NEG) == 0 for spacer/pad color bias


def _round_fp32r(a):
    b = np.asarray(a, np.float32).view(np.uint32).astype(np.uint64)
    r = (b + 0x7FF + ((b >> 12) & 1)) & 0xFFFFF000
    return r.astype(np.uint32).view(np.float32)


def _split_fp32r(a):
    a = np.asarray(a, np.float32)
    hi = _round_fp32r(a)
    lo = _round_fp32r(a - hi)
    return hi, lo


def _build_nc(gcap, lbs):
    """lbs: list of NBUCK bucket heights L_j (segment width is L_j + 1)."""
    import concourse.bacc as bacc
    import concourse.mybir as mybir
    from concourse.tile import TileContext

    f32 = mybir.dt.float32
    f32r = mybir.dt.float32r
    bf16 = mybir.dt.bfloat16
    AF = mybir.ActivationFunctionType
    OP = mybir.AluOpType

    chunks = []
    c0 = 0
    while c0 < gcap:
        cw = min(CW, gcap - c0)
        chunks.append((c0, cw))
        c0 += cw

    nc = bacc.Bacc("TRN2", target_bir_lowering=False, debug=False)
    rhs_d = nc.declare_dram_parameter("rhs", [KQ, gcap], f32r, isOutput=False)
    lhsT_d = nc.declare_dram_parameter("lhsT", [KQ, 256], f32r, isOutput=False)
    out_d = nc.declare_dram_parameter("out", [128, NT_CORE], f32, isOutput=True)

    with TileContext(nc) as tc:
        with (
            tc.tile_pool(name="const", bufs=1) as cpool,
            tc.tile_pool(name="ps", bufs=3, space="PSUM") as pspool,
        ):
            rhs_t = cpool.tile([KQ, gcap], f32r)
            lhsT_t = cpool.tile([KQ, 256], f32r)
            d1p_t = cpool.tile([128, gcap], bf16)
            abuf = cpool.tile([128, gcap], f32)
            acbuf = cpool.tile([128, gcap], bf16)
            ombuf = cpool.tile([128, gcap], bf16)
            sbufS = cpool.tile([128, gcap + 1], bf16)
            wcbuf = cpool.tile([128, gcap], bf16)
            outsb = cpool.tile([128, NT_CORE], f32)
            dummy = cpool.tile([1, 2], f32)

            # input DMAs: rhs split so chunk-0 matmul starts early
            nc.scalar.dma_start(lhsT_t[:], lhsT_d[:])
            r3 = [(0, CW), (CW, 4 * CW), (4 * CW, gcap)]
            for a, b in r3:
                b = min(b, gcap)
                if b > a:
                    nc.sync.dma_start(rhs_t[:, a:b], rhs_d[:, a:b])

            # d1p built on device: zeros, then 1.0 at each bucket's spacers
            nc.gpsimd.memset(dummy[:], 0.0)
            nc.gpsimd.memset(d1p_t[:], 0.0)
            off = 0
            for lb in lbs:
                seg = lb + 1
                ap3 = d1p_t[:, off : off + NSEG * seg].rearrange(
                    "p (s l) -> p s l", l=seg
                )
                nc.gpsimd.memset(ap3[:, :, 0:1], 1.0)
                off += NSEG * seg
            nc.gpsimd.memset(sbufS[:, 0:1], 0.0)

            # ACT exp-table preload during input DMA
            nc.scalar.activation(dummy[0:1, 0:1], dummy[0:1, 1:2], AF.Exp, bias=0.0)

            for ci, (c0, cw) in enumerate(chunks):
                sl = slice(c0, c0 + cw)
                psA = pspool.tile([128, cw], f32, name="psA")
                psB = pspool.tile([128, cw], f32, name="psB")
                nc.tensor.matmul(
                    psA[:], lhsT_t[:, 0:128], rhs_t[:, sl], start=True, stop=True
                )
                nc.tensor.matmul(
                    psB[:], lhsT_t[:, 128:256], rhs_t[:, sl], start=True, stop=True
                )
                nc.scalar.activation(abuf[:, sl], psA[:], AF.Exp, bias=0.0)
                nc.scalar.activation(acbuf[:, sl], psB[:], AF.Exp, bias=0.0)
                nc.gpsimd.tensor_scalar(
                    ombuf[:, sl], abuf[:, sl], -1.0, 1.0, OP.mult, OP.add
                )
                nc.vector.tensor_tensor_scan(
                    sbufS[:, c0 + 1 : c0 + cw + 1],
                    ombuf[:, sl],
                    d1p_t[:, sl],
                    0.0 if ci == 0 else sbufS[:, c0 : c0 + 1],
                    OP.mult,
                    OP.max,
                )
                nc.vector.tensor_mul(wcbuf[:, sl], acbuf[:, sl], sbufS[:, sl])

            off = 0
            for j, lb in enumerate(lbs):
                seg = lb + 1
                ap3 = wcbuf[:, off : off + NSEG * seg].rearrange(
                    "p (s l) -> p s l", l=seg
                )
                nc.vector.tensor_reduce(
                    outsb[:, j * NSEG : (j + 1) * NSEG],
                    ap3,
                    mybir.AxisListType.X,
                    OP.add,
                )
                off += NSEG * seg
            nc.sync.dma_start(out_d[:], outsb[:])

    nc.compile()
    return nc


_NC_CACHE = {}
LAST_RESULT = None


def _get_nc(gcap, lbs):
    key = (gcap, tuple(lbs))
    if key not in _NC_CACHE:
        _NC_CACHE[key] = _build_nc(gcap, lbs)
    return _NC_CACHE[key]


def _prep_inputs(means, quats, scales, rgbs, opacities):
    """Cull + pack per core. Returns (in_maps, tile_of, gcap, lbs)."""

    means = np.asarray(means, np.float64)
    quats = np.asarray(quats, np.float64)
    scales = np.asarray(scales, np.float64)
    rgbs = np.asarray(rgbs, np.float64)
    opacities = np.asarray(opacities, np.float64)

    c = np.cos(quats)
    s = np.sin(quats)
    sx2 = scales[:, 0] ** 2
    sy2 = scales[:, 1] ** 2
    a11 = c * c * sx2 + s * s * sy2
    a12 = c * s * (sx2 - sy2)
    a22 = s * s * sx2 + c * c * sy2
    det = a11 * a22 - a12 * a12
    ia = a22 / det
    ib = -a12 / det
    ic = a11 / det
    logopac = -np.logaddexp(0.0, -opacities)
    colors = 1.0 / (1.0 + np.exp(-rgbs[:, 0]))
    lnc = np.log(colors)
    rx = SIGMA_K * np.sqrt(a11)
    ry = SIGMA_K * np.sqrt(a22)
    x0g, x1g = means[:, 0] - rx, means[:, 0] + rx
    y0g, y1g = means[:, 1] - ry, means[:, 1] + ry

    tile_idx = []
    for t in range(NTILES):
        ty, tx = divmod(t, NTX)
        X0, X1 = tx * TW, (tx + 1) * TW
        Y0, Y1 = ty * TH, (ty + 1) * TH
        idx = np.nonzero(
            (x1g >= X0) & (x0g <= X1) & (y1g >= Y0) & (y0g <= Y1)
        )[0]
        tile_idx.append(idx)

    # global sort by count desc; rank r -> core r%8, position r//8
    order = sorted(range(NTILES), key=lambda t: -len(tile_idx[t]))
    # bucket heights: max count within each rank window of 64
    lbs = [len(tile_idx[order[64 * j]]) for j in range(NBUCK)]
    gcap = sum(NSEG * (lb + 1) for lb in lbs)

    fx = (np.arange(128) % TW).astype(np.float64) - (TW - 1) / 2.0
    fy = (np.arange(128) // TW).astype(np.float64) - (TH - 1) / 2.0
    basis5 = _round_fp32r(np.stack([fx * fx, fx * fy, fy * fy, fx, fy]))
    lhsT = np.zeros((KQ, 256), np.float32)
    lhsT[0:5, 0:128] = basis5
    lhsT[5, 0:128] = 1.0
    lhsT[6, 0:128] = 1.0
    lhsT[0:5, 128:256] = basis5
    lhsT[7, 128:256] = 1.0
    lhsT[8, 128:256] = 1.0

    in_maps = []
    tile_of = np.zeros((N_CORES, NT_CORE), np.int64)
    for core in range(N_CORES):
        rhs = np.zeros((KQ, gcap), np.float32)
        rhs[7, :] = NEG  # default color-bias: exp -> 0 at spacers/pads
        col = 0
        for p in range(NT_CORE):
            j = p // NSEG
            t = order[8 * p + core]
            tile_of[core, p] = t
            idx = tile_idx[t]
            k = len(idx)
            seg = lbs[j] + 1
            base = col + 1   # after spacer
            if k:
                ty, tx = divmod(t, NTX)
                cx = tx * TW + TW / 2.0
                cy = ty * TH + TH / 2.0
                mx = means[idx, 0] - cx
                my = means[idx, 1] - cy
                iag, ibg, icg = ia[idx], ib[idx], ic[idx]
                rhs[0, base : base + k] = _round_fp32r(-0.5 * iag)
                rhs[1, base : base + k] = _round_fp32r(-ibg)
                rhs[2, base : base + k] = _round_fp32r(-0.5 * icg)
                rhs[3, base : base + k] = _round_fp32r(iag * mx + ibg * my)
                rhs[4, base : base + k] = _round_fp32r(ibg * mx + icg * my)
                bias = logopac[idx] - 0.5 * (
                    iag * mx * mx + 2 * ibg * mx * my + icg * my * my
                )
                bh, bl = _split_fp32r(bias)
                bch, bcl = _split_fp32r(bias + lnc[idx])
                rhs[5, base : base + k] = bh
                rhs[6, base : base + k] = bl
                rhs[7, base : base + k] = bch
                rhs[8, base : base + k] = bcl
            col += seg
        in_maps.append({"rhs": rhs, "lhsT": lhsT})
    return in_maps, tile_of, gcap, lbs


def _assemble(results, tile_of):
    img = np.zeros((H, W), np.float32)
    for core in range(N_CORES):
        out = np.asarray(results[core]["out"], np.float32)  # [128, NT_CORE]
        for p in range(NT_CORE):
            t = tile_of[core, p]
            ty, tx = divmod(t, NTX)
            img[ty * TH : (ty + 1) * TH, tx * TW : (tx + 1) * TW] = out[
                :, p
            ].reshape(TH, TW)
    return img[None, None].astype(np.float32)


def kernel(means, quats, scales, rgbs, opacities):
    global LAST_RESULT
    from concourse.bass_utils import run_bass_kernel_spmd

    in_maps, tile_of, gcap, lbs = _prep_inputs(means, quats, scales, rgbs, opacities)
    nc = _get_nc(gcap, lbs)
    res = run_bass_kernel_spmd(nc, in_maps, list(range(N_CORES)))
    LAST_RESULT = res
    return _assemble(res.results, tile_of)
